# revision 2
# baseline (speedup 1.0000x reference)
"""Causal single-head attention on 8 TRN2 NeuronCores.

Problem (hardcoded): x [4, 2048, 1024] f32; Wk, Wq, Wv [1024, 1024] f32.
  q = x @ Wk.T ; k = x @ Wq.T ; v = x @ Wv.T        (note ref's q/k weight swap)
  out = softmax(mask(q @ k.T) / sqrt(1024)) @ v

Sharding: 2 cores per batch. Core h of a batch computes 1024 queries as two
512-query tiles: tile A with a 1024-key context, tile B with a 2048-key
context.  h=0 owns query blocks [0:512) + [1536:2048), h=1 owns [512:1536)
— every core runs the identical program (true SPMD); causality and padding
are encoded in per-core additive mask inputs.

On-chip layout is feature-major (all host-side transposes are free):
  xT/wT in, Q^T/K^T feature-major, V sequence-major.  Scores are computed
  as S^T[k, q] so softmax needs no on-chip transpose anywhere:
  exp via ACT (no max subtraction -- scaled scores are ~N(0,1), exp is
  safe in fp32), sum-of-exp via a ones-column matmul, AV accumulates
  out^T[e, q] with V as the stationary operand.  Output returns as out^T
  and is transposed back on the host.  All matmuls bf16 with fp32 PSUM.
"""

import functools

import ml_dtypes
import numpy as np

B = 4
S = 2048
D = 1024
P = 128
DCH = D // P            # 8 contraction chunks
QT = 512                # query-tile width
CTX_A, CTX_B = 1024, 2048
NKA, NKB = CTX_A // P, CTX_B // P
NEG = np.float32(-30000.0)

_BF16 = ml_dtypes.bfloat16


@functools.lru_cache(maxsize=1)
def _build_nc():
    import concourse.bass as bass  # noqa: F401  (registers engines)
    import concourse.mybir as mybir
    from concourse import bacc, tile

    bf16 = mybir.dt.bfloat16
    f32 = mybir.dt.float32
    add = mybir.AluOpType.add
    mult = mybir.AluOpType.mult
    Exp = mybir.ActivationFunctionType.Exp

    nc = bacc.Bacc("TRN2", target_bir_lowering=False, debug=False, num_devices=8)

    xT = nc.declare_dram_parameter("xT", [D, S], bf16, isOutput=False)
    xqT = nc.declare_dram_parameter("xqT", [D, 2 * QT], bf16, isOutput=False)
    wqT = nc.declare_dram_parameter("wqT", [D, D], bf16, isOutput=False)
    wkT = nc.declare_dram_parameter("wkT", [D, D], bf16, isOutput=False)
    wvT = nc.declare_dram_parameter("wvT", [D, D], bf16, isOutput=False)
    maskA = nc.declare_dram_parameter("maskA", [CTX_A, QT], f32, isOutput=False)
    maskB = nc.declare_dram_parameter("maskB", [CTX_B, QT], f32, isOutput=False)
    outT = nc.declare_dram_parameter("outT", [D, 2 * QT], f32, isOutput=True)

    with tile.TileContext(nc) as tc:
        with (
            tc.tile_pool(name="kv", bufs=1) as kv,
            tc.tile_pool(name="psum", bufs=2, space="PSUM") as psum,
        ):
            # ---- persistent SBUF tensors --------------------------------
            kt_sb = [kv.tile([P, S], bf16, tag=f"kt{e}", name=f"kt{e}") for e in range(DCH)]
            qt_sb = [kv.tile([P, 2 * QT], bf16, tag=f"qt{e}", name=f"qt{e}") for e in range(DCH)]
            v_sb = [kv.tile([P, D], bf16, tag=f"v{t}", name=f"v{t}") for t in range(S // P)]
            ones_sb = kv.tile([P, 1], bf16, tag="ones", name="ones")
            nc.gpsimd.memset(ones_sb[:], 1.0)

            # ---- phase 1: load inputs + QKV projections -----------------
            with tc.tile_pool(name="inp", bufs=1) as inp:
                x_sb = [inp.tile([P, S], bf16, tag=f"x{d}", name=f"x{d}") for d in range(DCH)]
                xq_sb = [inp.tile([P, 2 * QT], bf16, tag=f"xq{d}", name=f"xq{d}") for d in range(DCH)]
                wq_sb = [inp.tile([P, D], bf16, tag=f"wq{d}", name=f"wq{d}") for d in range(DCH)]
                wk_sb = [inp.tile([P, D], bf16, tag=f"wk{d}", name=f"wk{d}") for d in range(DCH)]
                wv_sb = [inp.tile([P, D], bf16, tag=f"wv{d}", name=f"wv{d}") for d in range(DCH)]
                for d in range(DCH):
                    rows = slice(d * P, (d + 1) * P)
                    nc.sync.dma_start(out=wk_sb[d][:], in_=wkT[rows, :])
                    nc.sync.dma_start(out=x_sb[d][:], in_=xT[rows, :])
                    nc.sync.dma_start(out=wq_sb[d][:], in_=wqT[rows, :])
                    nc.sync.dma_start(out=xq_sb[d][:], in_=xqT[rows, :])
                    nc.sync.dma_start(out=wv_sb[d][:], in_=wvT[rows, :])

                # K^T[e, t] feature-major, built 512 key-columns at a time
                for lc in range(S // QT):
                    lsl = slice(lc * QT, (lc + 1) * QT)
                    for e in range(DCH):
                        esl = slice(e * P, (e + 1) * P)
                        ps = psum.tile([P, QT], f32, tag="proj")
                        for d in range(DCH):
                            nc.tensor.matmul(
                                ps[:], wk_sb[d][:, esl], x_sb[d][:, lsl],
                                start=(d == 0), stop=(d == DCH - 1),
                            )
                        nc.vector.tensor_copy(kt_sb[e][:, lsl], ps[:])

                # Q^T[e, q] for this core's 1024 query columns
                for qh in range(2):
                    qsl = slice(qh * QT, (qh + 1) * QT)
                    for e in range(DCH):
                        esl = slice(e * P, (e + 1) * P)
                        ps = psum.tile([P, QT], f32, tag="proj")
                        for d in range(DCH):
                            nc.tensor.matmul(
                                ps[:], wq_sb[d][:, esl], xq_sb[d][:, qsl],
                                start=(d == 0), stop=(d == DCH - 1),
                            )
                        nc.vector.tensor_copy(qt_sb[e][:, qsl], ps[:])

                # V[t, e] sequence-major
                for t in range(S // P):
                    tsl = slice(t * P, (t + 1) * P)
                    for eh in range(2):
                        esl = slice(eh * QT, (eh + 1) * QT)
                        ps = psum.tile([P, QT], f32, tag="proj")
                        for d in range(DCH):
                            nc.tensor.matmul(
                                ps[:], x_sb[d][:, tsl], wv_sb[d][:, esl],
                                start=(d == 0), stop=(d == DCH - 1),
                            )
                        nc.vector.tensor_copy(v_sb[t][:, esl], ps[:])

            # ---- phase 2: attention, one 512-query tile at a time -------
            with (
                tc.tile_pool(name="pp", bufs=2) as pp,
                tc.tile_pool(name="mk", bufs=4) as mkp,
                tc.tile_pool(name="ost", bufs=4) as ost,
                tc.tile_pool(name="msc", bufs=2) as msc,
            ):
                for t_idx, (nk, mask_d) in enumerate(((NKA, maskA), (NKB, maskB))):
                    qsl = slice(t_idx * QT, (t_idx + 1) * QT)
                    phat = []
                    for k in range(nk):
                        mk = mkp.tile([P, QT], f32, tag="mask")
                        nc.sync.dma_start(
                            out=mk[:], in_=mask_d[k * P:(k + 1) * P, :]
                        )
                        ps = psum.tile([P, QT], f32, tag="sc")
                        ksl = slice(k * P, (k + 1) * P)
                        for e in range(DCH):
                            nc.tensor.matmul(
                                ps[:], kt_sb[e][:, ksl], qt_sb[e][:, qsl],
                                start=(e == 0), stop=(e == DCH - 1),
                            )
                        nc.vector.tensor_tensor(ps[:], ps[:], mk[:], op=add)
                        ph = pp.tile([P, QT], bf16, tag=f"p{k}")
                        # exp((score + mask) / sqrt(1024)); no max subtraction
                        nc.scalar.activation(ph[:], ps[:], Exp, scale=0.03125)
                        phat.append(ph)

                    sps = psum.tile([1, QT], f32, tag="sum")
                    for k in range(nk):
                        nc.tensor.matmul(
                            sps[:], ones_sb[:, 0:1], phat[k][:],
                            start=(k == 0), stop=(k == nk - 1),
                        )
                    rec = msc.tile([1, QT], f32, tag="rec")
                    nc.vector.reciprocal(rec[:], sps[:])
                    recb = msc.tile([P, QT], f32, tag="recb")
                    nc.gpsimd.partition_broadcast(recb[:], rec[:])

                    for e in range(DCH):
                        esl = slice(e * P, (e + 1) * P)
                        ps = psum.tile([P, QT], f32, tag="av")
                        for k in range(nk):
                            nc.tensor.matmul(
                                ps[:], v_sb[k][:, esl], phat[k][:],
                                start=(k == 0), stop=(k == nk - 1),
                            )
                        ot = ost.tile([P, QT], f32, tag="ot")
                        nc.vector.tensor_tensor(ot[:], ps[:], recb[:], op=mult)
                        nc.sync.dma_start(out=outT[esl, qsl], in_=ot[:])

    nc.compile()
    return nc


# h=0 -> query blocks [0:512) (tile A) and [1536:2048) (tile B)
# h=1 -> query blocks [512:1024) (tile A) and [1024:1536) (tile B)
_QSTARTS = ((0, 3 * QT), (QT, 2 * QT))


def _make_mask(q0: int, ctx: int) -> np.ndarray:
    k = np.arange(ctx)[:, None]
    q = q0 + np.arange(QT)[None, :]
    return np.where(k <= q, np.float32(0.0), NEG).astype(np.float32)


def _in_maps(x, Wk, Wq, Wv):
    wq_t = np.ascontiguousarray(Wk.T.astype(_BF16))   # ref swap: q uses Wk
    wk_t = np.ascontiguousarray(Wq.T.astype(_BF16))
    wv_t = np.ascontiguousarray(Wv.T.astype(_BF16))
    maps = []
    for c in range(8):
        b, h = divmod(c, 2)
        qa, qb = _QSTARTS[h]
        xb = x[b].astype(_BF16)
        x_t = np.ascontiguousarray(xb.T)
        xq_t = np.ascontiguousarray(
            np.concatenate([xb[qa:qa + QT], xb[qb:qb + QT]], axis=0).T
        )
        maps.append({
            "xT": x_t,
            "xqT": xq_t,
            "wqT": wq_t,
            "wkT": wk_t,
            "wvT": wv_t,
            "maskA": _make_mask(qa, CTX_A),
            "maskB": _make_mask(qb, CTX_B),
        })
    return maps


def _assemble(results):
    out = np.empty((B, S, D), dtype=np.float32)
    for c, res in enumerate(results):
        b, h = divmod(c, 2)
        qa, qb = _QSTARTS[h]
        o = res["outT"]
        out[b, qa:qa + QT] = o[:, 0:QT].T
        out[b, qb:qb + QT] = o[:, QT:2 * QT].T
    return out


def kernel(x, Wk, Wq, Wv, _trace=False):
    from concourse.bass_utils import run_bass_kernel_spmd

    nc = _build_nc()
    res = run_bass_kernel_spmd(nc, _in_maps(x, Wk, Wq, Wv), list(range(8)),
                               trace=_trace)
    out = _assemble(res.results)
    if _trace:
        return out, res
    return out


# revision 4
# speedup vs baseline: 1.2768x; 1.2768x over previous
"""Causal single-head attention on 8 TRN2 NeuronCores.

Problem (hardcoded): x [4, 2048, 1024] f32; Wk, Wq, Wv [1024, 1024] f32.
  q = x @ Wk.T ; k = x @ Wq.T ; v = x @ Wv.T        (note ref's q/k weight swap)
  out = softmax(mask(q @ k.T) / sqrt(1024)) @ v

Sharding: 2 cores per batch. Core h of a batch computes 1024 queries as two
512-query tiles: tile A with a 1024-key context, tile B with a 2048-key
context.  h=0 owns query blocks [0:512) + [1536:2048), h=1 owns [512:1536)
— every core runs the identical program (true SPMD); causality and padding
are encoded in per-core additive mask inputs.

On-chip layout is feature-major (all host-side transposes are free):
  xT/wT in, Q^T/K^T feature-major, V sequence-major.  Scores are computed
  as S^T[k, q] so softmax needs no on-chip transpose anywhere:
  exp via ACT (no max subtraction -- scaled scores are ~N(0,1), exp is
  safe in fp32), sum-of-exp via a ones-column matmul, AV accumulates
  out^T[e, q] with V as the stationary operand.  Output returns as out^T
  and is transposed back on the host.  All matmuls bf16 with fp32 PSUM.

Projection loops are weight-stationary: one LDWEIGHTS feeds 4 (K-proj) or
2 (Q/V-proj) matmuls into parallel PSUM banks, cutting PE issue overhead.
"""

import functools

import ml_dtypes
import numpy as np

B = 4
S = 2048
D = 1024
P = 128
DCH = D // P            # 8 contraction chunks
QT = 512                # query-tile width
CTX_A, CTX_B = 1024, 2048
NKA, NKB = CTX_A // P, CTX_B // P
NEG = np.float32(-30000.0)

_BF16 = ml_dtypes.bfloat16


@functools.lru_cache(maxsize=1)
def _build_nc():
    import concourse.bass as bass  # noqa: F401  (registers engines)
    import concourse.mybir as mybir
    from concourse import bacc, tile

    bf16 = mybir.dt.bfloat16
    f32 = mybir.dt.float32
    add = mybir.AluOpType.add
    mult = mybir.AluOpType.mult
    Exp = mybir.ActivationFunctionType.Exp

    nc = bacc.Bacc("TRN2", target_bir_lowering=False, debug=False, num_devices=8)

    xT = nc.declare_dram_parameter("xT", [D, S], bf16, isOutput=False)
    xqT = nc.declare_dram_parameter("xqT", [D, 2 * QT], bf16, isOutput=False)
    wqT = nc.declare_dram_parameter("wqT", [D, D], bf16, isOutput=False)
    wkT = nc.declare_dram_parameter("wkT", [D, D], bf16, isOutput=False)
    wvT = nc.declare_dram_parameter("wvT", [D, D], bf16, isOutput=False)
    maskA = nc.declare_dram_parameter("maskA", [CTX_A, QT], f32, isOutput=False)
    maskB = nc.declare_dram_parameter("maskB", [CTX_B, QT], f32, isOutput=False)
    outT = nc.declare_dram_parameter("outT", [D, 2 * QT], f32, isOutput=True)

    with tile.TileContext(nc) as tc:
        with tc.tile_pool(name="kv", bufs=1) as kv:
            # ---- persistent SBUF tensors --------------------------------
            kt_sb = [kv.tile([P, S], bf16, tag=f"kt{e}", name=f"kt{e}")
                     for e in range(DCH)]
            qt_sb = [kv.tile([P, 2 * QT], bf16, tag=f"qt{e}", name=f"qt{e}")
                     for e in range(DCH)]
            v_sb = [kv.tile([P, D], bf16, tag=f"v{t}", name=f"v{t}")
                    for t in range(S // P)]
            ones_sb = kv.tile([P, 1], bf16, tag="ones", name="ones")
            nc.gpsimd.memset(ones_sb[:], 1.0)

            # ---- phase 1: load inputs + QKV projections -----------------
            with (
                tc.tile_pool(name="inp", bufs=1) as inp,
                tc.tile_pool(name="pps", bufs=2, space="PSUM") as pps,
            ):
                x_sb = [inp.tile([P, S], bf16, tag=f"x{d}", name=f"x{d}")
                        for d in range(DCH)]
                xq_sb = [inp.tile([P, 2 * QT], bf16, tag=f"xq{d}", name=f"xq{d}")
                         for d in range(DCH)]
                wq_sb = [inp.tile([P, D], bf16, tag=f"wq{d}", name=f"wq{d}")
                         for d in range(DCH)]
                wk_sb = [inp.tile([P, D], bf16, tag=f"wk{d}", name=f"wk{d}")
                         for d in range(DCH)]
                wv_sb = [inp.tile([P, D], bf16, tag=f"wv{d}", name=f"wv{d}")
                        for d in range(DCH)]
                # Load order = first-use order; x is split per 512-column
                # chunk so the first K-proj group isn't gated on all of x.
                for d in range(DCH):
                    rows = slice(d * P, (d + 1) * P)
                    nc.sync.dma_start(out=wk_sb[d][:], in_=wkT[rows, :])
                for lc in range(S // QT):
                    lsl = slice(lc * QT, (lc + 1) * QT)
                    for d in range(DCH):
                        rows = slice(d * P, (d + 1) * P)
                        nc.sync.dma_start(out=x_sb[d][:, lsl], in_=xT[rows, lsl])
                for d in range(DCH):
                    rows = slice(d * P, (d + 1) * P)
                    nc.sync.dma_start(out=wq_sb[d][:], in_=wqT[rows, :])
                    nc.sync.dma_start(out=xq_sb[d][:], in_=xqT[rows, :])
                    nc.sync.dma_start(out=wv_sb[d][:], in_=wvT[rows, :])

                # K^T[e, t] feature-major. Weight-stationary: one wk weight
                # tile drives 4 matmuls into 4 parallel PSUM banks.
                for e in range(DCH):
                    esl = slice(e * P, (e + 1) * P)
                    pss = [pps.tile([P, QT], f32, tag=f"pj{lc}", name=f"kps{e}_{lc}")
                           for lc in range(4)]
                    for d in range(DCH):
                        for lc in range(4):
                            nc.tensor.matmul(
                                pss[lc][:], wk_sb[d][:, esl],
                                x_sb[d][:, lc * QT:(lc + 1) * QT],
                                start=(d == 0), stop=(d == DCH - 1),
                            )
                    for lc in range(4):
                        nc.vector.tensor_copy(
                            kt_sb[e][:, lc * QT:(lc + 1) * QT], pss[lc][:])

                # Q^T[e, q]: one wq weight tile drives both query halves.
                for e in range(DCH):
                    esl = slice(e * P, (e + 1) * P)
                    pss = [pps.tile([P, QT], f32, tag=f"pj{qh}", name=f"qps{e}_{qh}")
                           for qh in range(2)]
                    for d in range(DCH):
                        for qh in range(2):
                            nc.tensor.matmul(
                                pss[qh][:], wq_sb[d][:, esl],
                                xq_sb[d][:, qh * QT:(qh + 1) * QT],
                                start=(d == 0), stop=(d == DCH - 1),
                            )
                    for qh in range(2):
                        nc.vector.tensor_copy(
                            qt_sb[e][:, qh * QT:(qh + 1) * QT], pss[qh][:])

                # V[t, e] sequence-major: one x weight tile drives both
                # output-feature halves.
                for t in range(S // P):
                    tsl = slice(t * P, (t + 1) * P)
                    pss = [pps.tile([P, QT], f32, tag=f"pj{eh}", name=f"vps{t}_{eh}")
                           for eh in range(2)]
                    for d in range(DCH):
                        for eh in range(2):
                            nc.tensor.matmul(
                                pss[eh][:], x_sb[d][:, tsl],
                                wv_sb[d][:, eh * QT:(eh + 1) * QT],
                                start=(d == 0), stop=(d == DCH - 1),
                            )
                    for eh in range(2):
                        nc.vector.tensor_copy(
                            v_sb[t][:, eh * QT:(eh + 1) * QT], pss[eh][:])

            # ---- phase 2: attention, one 512-query tile at a time -------
            with (
                tc.tile_pool(name="pp", bufs=2) as pp,
                tc.tile_pool(name="mk", bufs=6) as mkp,
                tc.tile_pool(name="ost", bufs=4) as ost,
                tc.tile_pool(name="msc", bufs=2) as msc,
                tc.tile_pool(name="aps", bufs=2, space="PSUM") as aps,
            ):
                for t_idx, (nk, mask_d) in enumerate(((NKA, maskA), (NKB, maskB))):
                    qsl = slice(t_idx * QT, (t_idx + 1) * QT)
                    phat = []
                    for k in range(nk):
                        mk = mkp.tile([P, QT], f32, tag="mask", name=f"m{t_idx}_{k}")
                        nc.sync.dma_start(
                            out=mk[:], in_=mask_d[k * P:(k + 1) * P, :]
                        )
                        ps = aps.tile([P, QT], f32, tag="sc", name=f"sc{t_idx}_{k}")
                        ksl = slice(k * P, (k + 1) * P)
                        for e in range(DCH):
                            nc.tensor.matmul(
                                ps[:], kt_sb[e][:, ksl], qt_sb[e][:, qsl],
                                start=(e == 0), stop=(e == DCH - 1),
                            )
                        nc.vector.tensor_tensor(ps[:], ps[:], mk[:], op=add)
                        ph = pp.tile([P, QT], bf16, tag=f"p{k}", name=f"ph{t_idx}_{k}")
                        # exp((score + mask) / sqrt(1024)); no max subtraction
                        nc.scalar.activation(ph[:], ps[:], Exp, scale=0.03125)
                        phat.append(ph)

                    sps = aps.tile([1, QT], f32, tag="sum", name=f"sum{t_idx}")
                    for k in range(nk):
                        nc.tensor.matmul(
                            sps[:], ones_sb[:, 0:1], phat[k][:],
                            start=(k == 0), stop=(k == nk - 1),
                        )
                    # broadcast then full-width reciprocal (a [1,512]
                    # reciprocal runs on a single DVE lane -- slow);
                    # GPSIMD can't read PSUM, so stage the row in SBUF.
                    srow = msc.tile([1, QT], f32, tag="srow", name=f"srow{t_idx}")
                    nc.vector.tensor_copy(srow[:], sps[:])
                    sumb = msc.tile([P, QT], f32, tag="sumb", name=f"sumb{t_idx}")
                    nc.gpsimd.partition_broadcast(sumb[:], srow[:])
                    recb = msc.tile([P, QT], f32, tag="recb", name=f"recb{t_idx}")
                    nc.vector.reciprocal(recb[:], sumb[:])

                    for e in range(DCH):
                        esl = slice(e * P, (e + 1) * P)
                        ps = aps.tile([P, QT], f32, tag="av", name=f"av{t_idx}_{e}")
                        for k in range(nk):
                            nc.tensor.matmul(
                                ps[:], v_sb[k][:, esl], phat[k][:],
                                start=(k == 0), stop=(k == nk - 1),
                            )
                        ot = ost.tile([P, QT], f32, tag="ot", name=f"ot{t_idx}_{e}")
                        nc.vector.tensor_tensor(ot[:], ps[:], recb[:], op=mult)
                        nc.sync.dma_start(out=outT[esl, qsl], in_=ot[:])

    nc.compile()
    return nc


# h=0 -> query blocks [0:512) (tile A) and [1536:2048) (tile B)
# h=1 -> query blocks [512:1024) (tile A) and [1024:1536) (tile B)
_QSTARTS = ((0, 3 * QT), (QT, 2 * QT))


def _make_mask(q0: int, ctx: int) -> np.ndarray:
    k = np.arange(ctx)[:, None]
    q = q0 + np.arange(QT)[None, :]
    return np.where(k <= q, np.float32(0.0), NEG).astype(np.float32)


def _in_maps(x, Wk, Wq, Wv):
    wq_t = np.ascontiguousarray(Wk.T.astype(_BF16))   # ref swap: q uses Wk
    wk_t = np.ascontiguousarray(Wq.T.astype(_BF16))
    wv_t = np.ascontiguousarray(Wv.T.astype(_BF16))
    maps = []
    for c in range(8):
        b, h = divmod(c, 2)
        qa, qb = _QSTARTS[h]
        xb = x[b].astype(_BF16)
        x_t = np.ascontiguousarray(xb.T)
        xq_t = np.ascontiguousarray(
            np.concatenate([xb[qa:qa + QT], xb[qb:qb + QT]], axis=0).T
        )
        maps.append({
            "xT": x_t,
            "xqT": xq_t,
            "wqT": wq_t,
            "wkT": wk_t,
            "wvT": wv_t,
            "maskA": _make_mask(qa, CTX_A),
            "maskB": _make_mask(qb, CTX_B),
        })
    return maps


def _assemble(results):
    out = np.empty((B, S, D), dtype=np.float32)
    for c, res in enumerate(results):
        b, h = divmod(c, 2)
        qa, qb = _QSTARTS[h]
        o = res["outT"]
        out[b, qa:qa + QT] = o[:, 0:QT].T
        out[b, qb:qb + QT] = o[:, QT:2 * QT].T
    return out


def kernel(x, Wk, Wq, Wv, _trace=False):
    from concourse.bass_utils import run_bass_kernel_spmd

    nc = _build_nc()
    res = run_bass_kernel_spmd(nc, _in_maps(x, Wk, Wq, Wv), list(range(8)),
                               trace=_trace)
    out = _assemble(res.results)
    if _trace:
        return out, res
    return out


# revision 6
# speedup vs baseline: 1.4081x; 1.1028x over previous
"""Causal single-head attention on 8 TRN2 NeuronCores.

Problem (hardcoded): x [4, 2048, 1024] f32; Wk, Wq, Wv [1024, 1024] f32.
  q = x @ Wk.T ; k = x @ Wq.T ; v = x @ Wv.T        (note ref's q/k weight swap)
  out = softmax(mask(q @ k.T) / sqrt(1024)) @ v

Sharding: 2 cores per batch. Core h of a batch computes 1024 queries as two
512-query tiles: tile A with a 1024-key context, tile B with a 2048-key
context.  h=0 owns query blocks [0:512) + [1536:2048), h=1 owns [512:1536)
— every core runs the identical program (true SPMD); causality and padding
are encoded in per-core additive mask inputs.

K/V for keys [0:1024) are computed on both cores of a pair (both need them
immediately for tile A).  K/V for keys [1024:2048) are split: each core
projects only the keys its host put at xT columns [1024:1536) (h=0: keys
[1024:1536), h=1: keys [1536:2048)), and the halves are exchanged with a
pair AllGather through DRAM bounce buffers.  Both ranks read back BOTH
gathered regions (region r holds group-rank r's quarter), which lands the
exchanged data in identical key order on both ranks — masks stay standard
causal and nothing else is permuted.

On-chip layout is feature-major (all host-side transposes are free):
  xT/wT in, Q^T/K^T feature-major, V sequence-major.  Scores are computed
  as S^T[k, q] so softmax needs no on-chip transpose anywhere:
  exp via ACT (no max subtraction -- scaled scores are ~N(0,1), exp is
  safe in fp32), sum-of-exp via a ones-column matmul, AV accumulates
  out^T[e, q] with V as the stationary operand.  Output returns as out^T
  and is transposed back on the host.  All matmuls bf16 with fp32 PSUM.

Projection loops are weight-stationary: one LDWEIGHTS feeds several
matmuls into parallel PSUM banks, cutting PE issue overhead.
"""

import functools

import ml_dtypes
import numpy as np

B = 4
S = 2048
D = 1024
P = 128
DCH = D // P            # 8 contraction chunks
QT = 512                # query-tile width
CTX_A, CTX_B = 1024, 2048
NKA, NKB = CTX_A // P, CTX_B // P
NEG = np.float32(-30000.0)

_BF16 = ml_dtypes.bfloat16


@functools.lru_cache(maxsize=1)
def _build_nc():
    import concourse.bass as bass  # noqa: F401  (registers engines)
    import concourse.mybir as mybir
    from concourse import bacc, tile

    bf16 = mybir.dt.bfloat16
    f32 = mybir.dt.float32
    add = mybir.AluOpType.add
    mult = mybir.AluOpType.mult
    Exp = mybir.ActivationFunctionType.Exp
    PAIRS = [[2 * i, 2 * i + 1] for i in range(4)]

    nc = bacc.Bacc("TRN2", target_bir_lowering=False, debug=False, num_devices=8)

    xT = nc.declare_dram_parameter("xT", [D, S], bf16, isOutput=False)
    xqT = nc.declare_dram_parameter("xqT", [D, 2 * QT], bf16, isOutput=False)
    wqT = nc.declare_dram_parameter("wqT", [D, D], bf16, isOutput=False)
    wkT = nc.declare_dram_parameter("wkT", [D, D], bf16, isOutput=False)
    wvT = nc.declare_dram_parameter("wvT", [D, D], bf16, isOutput=False)
    maskA = nc.declare_dram_parameter("maskA", [CTX_A, QT], f32, isOutput=False)
    maskB = nc.declare_dram_parameter("maskB", [CTX_B, QT], f32, isOutput=False)
    outT = nc.declare_dram_parameter("outT", [D, 2 * QT], f32, isOutput=True)

    with tile.TileContext(nc) as tc:
        with (
            tc.tile_pool(name="kv", bufs=1) as kv,
            tc.tile_pool(name="dram", bufs=1, space="DRAM") as dram,
        ):
            # ---- persistent SBUF tensors --------------------------------
            kt_sb = [kv.tile([P, S], bf16, tag=f"kt{e}", name=f"kt{e}")
                     for e in range(DCH)]
            qt_sb = [kv.tile([P, 2 * QT], bf16, tag=f"qt{e}", name=f"qt{e}")
                     for e in range(DCH)]
            v_sb = [kv.tile([P, D], bf16, tag=f"v{t}", name=f"v{t}")
                    for t in range(S // P)]
            ones_sb = kv.tile([P, 1], bf16, tag="ones", name="ones")
            nc.gpsimd.memset(ones_sb[:], 1.0)

            # DRAM bounce buffers for the pair K/V exchange
            agin_v = dram.tile([4 * P, D], bf16, name="agin_v")
            agout_v = dram.tile([8 * P, D], bf16, name="agout_v")
            agin_k = dram.tile([D, QT], bf16, name="agin_k")
            agout_k = dram.tile([2 * D, QT], bf16, name="agout_k")

            # ---- phase 1: load inputs + QKV projections -----------------
            with (
                tc.tile_pool(name="inp", bufs=1) as inp,
                tc.tile_pool(name="pps", bufs=2, space="PSUM") as pps,
            ):
                x_sb = [inp.tile([P, S], bf16, tag=f"x{d}", name=f"x{d}")
                        for d in range(DCH)]
                xq_sb = [inp.tile([P, 2 * QT], bf16, tag=f"xq{d}", name=f"xq{d}")
                         for d in range(DCH)]
                wq_sb = [inp.tile([P, D], bf16, tag=f"wq{d}", name=f"wq{d}")
                         for d in range(DCH)]
                wk_sb = [inp.tile([P, D], bf16, tag=f"wk{d}", name=f"wk{d}")
                         for d in range(DCH)]
                wv_sb = [inp.tile([P, D], bf16, tag=f"wv{d}", name=f"wv{d}")
                        for d in range(DCH)]
                # Load order = first-use order (V's exchanged quarter first
                # so the collective can start early).
                for d in range(DCH):
                    rows = slice(d * P, (d + 1) * P)
                    nc.sync.dma_start(out=wv_sb[d][:], in_=wvT[rows, :])
                for lc in (2, 0, 1):
                    lsl = slice(lc * QT, (lc + 1) * QT)
                    for d in range(DCH):
                        rows = slice(d * P, (d + 1) * P)
                        nc.sync.dma_start(out=x_sb[d][:, lsl], in_=xT[rows, lsl])
                for d in range(DCH):
                    rows = slice(d * P, (d + 1) * P)
                    nc.sync.dma_start(out=wk_sb[d][:], in_=wkT[rows, :])
                for d in range(DCH):
                    rows = slice(d * P, (d + 1) * P)
                    nc.sync.dma_start(out=wq_sb[d][:], in_=wqT[rows, :])
                    nc.sync.dma_start(out=xq_sb[d][:], in_=xqT[rows, :])

                def v_proj(t):
                    tsl = slice(t * P, (t + 1) * P)
                    pss = [pps.tile([P, QT], f32, tag=f"pj{eh}",
                                    name=f"vps{t}_{eh}") for eh in range(2)]
                    for d in range(DCH):
                        for eh in range(2):
                            nc.tensor.matmul(
                                pss[eh][:], x_sb[d][:, tsl],
                                wv_sb[d][:, eh * QT:(eh + 1) * QT],
                                start=(d == 0), stop=(d == DCH - 1),
                            )
                    for eh in range(2):
                        nc.vector.tensor_copy(
                            v_sb[t][:, eh * QT:(eh + 1) * QT], pss[eh][:])

                # V for the exchanged quarter first (positions 1024:1536)
                for t in range(8, 12):
                    v_proj(t)
                for t in range(8, 12):
                    nc.sync.dma_start(
                        out=agin_v[(t - 8) * P:(t - 7) * P, :], in_=v_sb[t][:])
                nc.gpsimd.collective_compute(
                    "AllGather", mybir.AluOpType.bypass,
                    replica_groups=PAIRS,
                    ins=[agin_v[:]], outs=[agout_v[:]],
                )
                # Read back BOTH regions: region r = group-rank r's quarter,
                # so the final key order is identical on both ranks.
                for t in range(8, 16):
                    nc.sync.dma_start(
                        out=v_sb[t][:], in_=agout_v[(t - 8) * P:(t - 7) * P, :])
                # V for keys [0:1024) (both cores need these for tile A)
                for t in range(8):
                    v_proj(t)

                # K^T feature-major. Weight-stationary: one wk weight tile
                # drives 3 matmuls (key cols 0:1536; 1536:2048 arrives via
                # the exchange) into parallel PSUM banks.
                for e in range(DCH):
                    esl = slice(e * P, (e + 1) * P)
                    pss = [pps.tile([P, QT], f32, tag=f"pj{i}",
                                    name=f"kps{e}_{i}") for i in range(3)]
                    for d in range(DCH):
                        for i, lc in enumerate((2, 0, 1)):
                            nc.tensor.matmul(
                                pss[i][:], wk_sb[d][:, esl],
                                x_sb[d][:, lc * QT:(lc + 1) * QT],
                                start=(d == 0), stop=(d == DCH - 1),
                            )
                    for i, lc in enumerate((2, 0, 1)):
                        nc.vector.tensor_copy(
                            kt_sb[e][:, lc * QT:(lc + 1) * QT], pss[i][:])
                    nc.sync.dma_start(
                        out=agin_k[e * P:(e + 1) * P, :],
                        in_=kt_sb[e][:, 2 * QT:3 * QT])
                nc.gpsimd.collective_compute(
                    "AllGather", mybir.AluOpType.bypass,
                    replica_groups=PAIRS,
                    ins=[agin_k[:]], outs=[agout_k[:]],
                )
                for e in range(DCH):
                    nc.sync.dma_start(
                        out=kt_sb[e][:, 2 * QT:3 * QT],
                        in_=agout_k[e * P:(e + 1) * P, :])
                    nc.sync.dma_start(
                        out=kt_sb[e][:, 3 * QT:4 * QT],
                        in_=agout_k[D + e * P:D + (e + 1) * P, :])

                # Q^T[e, q]: one wq weight tile drives both query halves.
                for e in range(DCH):
                    esl = slice(e * P, (e + 1) * P)
                    pss = [pps.tile([P, QT], f32, tag=f"pj{qh}",
                                    name=f"qps{e}_{qh}") for qh in range(2)]
                    for d in range(DCH):
                        for qh in range(2):
                            nc.tensor.matmul(
                                pss[qh][:], wq_sb[d][:, esl],
                                xq_sb[d][:, qh * QT:(qh + 1) * QT],
                                start=(d == 0), stop=(d == DCH - 1),
                            )
                    for qh in range(2):
                        nc.vector.tensor_copy(
                            qt_sb[e][:, qh * QT:(qh + 1) * QT], pss[qh][:])

            # ---- phase 2: attention, one 512-query tile at a time -------
            with (
                tc.tile_pool(name="pp", bufs=2) as pp,
                tc.tile_pool(name="mk", bufs=6) as mkp,
                tc.tile_pool(name="ost", bufs=4) as ost,
                tc.tile_pool(name="msc", bufs=2) as msc,
                tc.tile_pool(name="scp", bufs=4, space="PSUM") as scp,
                tc.tile_pool(name="aps", bufs=2, space="PSUM") as aps,
            ):
                for t_idx, (nk, mask_d) in enumerate(((NKA, maskA), (NKB, maskB))):
                    qsl = slice(t_idx * QT, (t_idx + 1) * QT)
                    phat = []
                    for k in range(nk):
                        mk = mkp.tile([P, QT], f32, tag="mask", name=f"m{t_idx}_{k}")
                        nc.sync.dma_start(
                            out=mk[:], in_=mask_d[k * P:(k + 1) * P, :]
                        )
                        ps = scp.tile([P, QT], f32, tag="sc", name=f"sc{t_idx}_{k}")
                        ksl = slice(k * P, (k + 1) * P)
                        for e in range(DCH):
                            nc.tensor.matmul(
                                ps[:], kt_sb[e][:, ksl], qt_sb[e][:, qsl],
                                start=(e == 0), stop=(e == DCH - 1),
                            )
                        nc.vector.tensor_tensor(ps[:], ps[:], mk[:], op=add)
                        ph = pp.tile([P, QT], bf16, tag=f"p{k}", name=f"ph{t_idx}_{k}")
                        # exp((score + mask) / sqrt(1024)); no max subtraction
                        nc.scalar.activation(ph[:], ps[:], Exp, scale=0.03125)
                        phat.append(ph)

                    sps = aps.tile([1, QT], f32, tag="sum", name=f"sum{t_idx}")
                    for k in range(nk):
                        nc.tensor.matmul(
                            sps[:], ones_sb[:, 0:1], phat[k][:],
                            start=(k == 0), stop=(k == nk - 1),
                        )
                    # broadcast then full-width fast reciprocal (a [1,512]
                    # reciprocal runs on a single DVE lane -- slow);
                    # GPSIMD can't read PSUM, so stage the row in SBUF.
                    srow = msc.tile([1, QT], f32, tag="srow", name=f"srow{t_idx}")
                    nc.vector.tensor_copy(srow[:], sps[:])
                    sumb = msc.tile([P, QT], f32, tag="sumb", name=f"sumb{t_idx}")
                    nc.gpsimd.partition_broadcast(sumb[:], srow[:])
                    recb = msc.tile([P, QT], f32, tag="recb", name=f"recb{t_idx}")
                    nc.vector.reciprocal_approx_fast(out=recb[:], in_=sumb[:])

                    for e in range(DCH):
                        esl = slice(e * P, (e + 1) * P)
                        ps = aps.tile([P, QT], f32, tag="av", name=f"av{t_idx}_{e}")
                        for k in range(nk):
                            nc.tensor.matmul(
                                ps[:], v_sb[k][:, esl], phat[k][:],
                                start=(k == 0), stop=(k == nk - 1),
                            )
                        ot = ost.tile([P, QT], f32, tag="ot", name=f"ot{t_idx}_{e}")
                        nc.vector.tensor_tensor(ot[:], ps[:], recb[:], op=mult)
                        nc.sync.dma_start(out=outT[esl, qsl], in_=ot[:])

    nc.compile()
    return nc


# h=0 -> query blocks [0:512) (tile A) and [1536:2048) (tile B)
# h=1 -> query blocks [512:1024) (tile A) and [1024:1536) (tile B)
_QSTARTS = ((0, 3 * QT), (QT, 2 * QT))


def _make_mask(q0: int, ctx: int) -> np.ndarray:
    k = np.arange(ctx)[:, None]
    q = q0 + np.arange(QT)[None, :]
    return np.where(k <= q, np.float32(0.0), NEG).astype(np.float32)


def _in_maps(x, Wk, Wq, Wv):
    wq_t = np.ascontiguousarray(Wk.T.astype(_BF16))   # ref swap: q uses Wk
    wk_t = np.ascontiguousarray(Wq.T.astype(_BF16))
    wv_t = np.ascontiguousarray(Wv.T.astype(_BF16))
    maps = []
    for c in range(8):
        b, h = divmod(c, 2)
        qa, qb = _QSTARTS[h]
        xb = x[b].astype(_BF16)
        xp = np.array(xb)
        if h == 1:
            # This core projects keys [1536:2048); its pair partner's
            # quarter lands back in identical key order via the exchange.
            xp[2 * QT:3 * QT] = xb[3 * QT:4 * QT]
        x_t = np.ascontiguousarray(xp.T)
        xq_t = np.ascontiguousarray(
            np.concatenate([xb[qa:qa + QT], xb[qb:qb + QT]], axis=0).T
        )
        maps.append({
            "xT": x_t,
            "xqT": xq_t,
            "wqT": wq_t,
            "wkT": wk_t,
            "wvT": wv_t,
            "maskA": _make_mask(qa, CTX_A),
            "maskB": _make_mask(qb, CTX_B),
        })
    return maps


def _assemble(results):
    out = np.empty((B, S, D), dtype=np.float32)
    for c, res in enumerate(results):
        b, h = divmod(c, 2)
        qa, qb = _QSTARTS[h]
        o = res["outT"]
        out[b, qa:qa + QT] = o[:, 0:QT].T
        out[b, qb:qb + QT] = o[:, QT:2 * QT].T
    return out


def kernel(x, Wk, Wq, Wv, _trace=False):
    from concourse.bass_utils import run_bass_kernel_spmd

    nc = _build_nc()
    res = run_bass_kernel_spmd(nc, _in_maps(x, Wk, Wq, Wv), list(range(8)),
                               trace=_trace)
    out = _assemble(res.results)
    if _trace:
        return out, res
    return out
